# revision 11
# baseline (speedup 1.0000x reference)
"""Distributed Trainium2 Bass kernel for a full causal attention layer.

Problem: B=2, S=2048, D_MODEL=1024, H=16, D_HEAD=64, causal + additive mask.

Sharding (8 cores): data-parallel over batch (cores 0-3 -> batch 0,
cores 4-7 -> batch 1) x tensor-parallel over heads (4 heads per core).
Each core:
  1. projects Q,K (transposed layout [head*dhead, seq]) and V (natural
     layout, with an extra ones-column per head) for its 4 heads,
  2. computes causal attention scores transposed S^T[k,q] = K @ Q^T,
     exp via ScalarE (additive mask folded in as per-partition bias,
     causal mask via a precomputed triangle tile on diagonal blocks,
     upper-triangle blocks skipped entirely),
  3. z_aug^T[65,q] = V_aug^T @ E accumulated over k tiles; row 64 is the
     softmax denominator. Normalizes via reciprocal + K=1 broadcast
     matmul.
  4. AllToAll within its 4-core group to reshard z^T from (all q, local
     heads) to (local 512 q rows, all 16 heads),
  5. output projection for its 512 q rows -> disjoint output row slices.
Host only transposes/shards inputs and concatenates the 8 output slices.
"""

import os
import sys

import numpy as np

for _p in ("/opt/trn_rl_repo", "/root/.axon_site/_ro/trn_rl_repo"):
    if os.path.isdir(_p) and _p not in sys.path:
        sys.path.insert(0, _p)

import concourse.bass as bass  # noqa: E402
import concourse.mybir as mybir  # noqa: E402
from concourse import bacc  # noqa: E402
from concourse import tile  # noqa: E402
from concourse.bass_utils import run_bass_kernel_spmd  # noqa: E402

F32 = mybir.dt.float32
F32R = mybir.dt.float32r

B, S, DM, H, DH = 2, 2048, 1024, 16, 64
N_CORES = 8
GROUP = 4              # cores per batch group
H_LOC = H // GROUP     # heads per core
WCOL = H_LOC * DH      # 256 projected cols per core
QR = S // GROUP        # 512 q rows owned per core after AllToAll
MASK_VAL = -1.0e5
SCALE = 1.0 / np.sqrt(DH).astype(np.float32)

DM_T = DM // 128       # 8 dmodel k-tiles
S_T = S // 128         # 16 seq 128-tiles
S_C = S // 512         # 4 seq 512-chunks


def build_bass():
    nc = bacc.Bacc("TRN2", target_bir_lowering=False, debug=False,
                   num_devices=N_CORES)

    xt_q = nc.dram_tensor("xt_q", [DM, S], F32R, kind="ExternalInput")
    xt_k = nc.dram_tensor("xt_k", [DM, S], F32R, kind="ExternalInput")
    xt_v = nc.dram_tensor("xt_v", [DM, S], F32R, kind="ExternalInput")
    w_q = nc.dram_tensor("w_q", [DM, WCOL], F32R, kind="ExternalInput")
    w_k = nc.dram_tensor("w_k", [DM, WCOL], F32R, kind="ExternalInput")
    w_v = nc.dram_tensor("w_v", [DM, WCOL], F32R, kind="ExternalInput")
    w_o = nc.dram_tensor("w_o", [DM, DM], F32R, kind="ExternalInput")
    bq = nc.dram_tensor("bq", [WCOL, 1], F32, kind="ExternalInput")
    bk = nc.dram_tensor("bk", [WCOL, 1], F32, kind="ExternalInput")
    bvb = nc.dram_tensor("bvb", [128, H_LOC * (DH + 1)], F32R, kind="ExternalInput")
    bob = nc.dram_tensor("bob", [128, DM], F32, kind="ExternalInput")
    maskt = nc.dram_tensor("maskt", [128, S_T], F32, kind="ExternalInput")
    tri = nc.dram_tensor("tri", [128, 896], F32, kind="ExternalInput")
    ones64 = nc.dram_tensor("ones64", [1, DH], F32R, kind="ExternalInput")
    out = nc.dram_tensor("out", [QR, DM], F32, kind="ExternalOutput")

    with tile.TileContext(nc) as tc:
        with (
            tc.tile_pool(name="persist", bufs=1) as pp,
            tc.tile_pool(name="xts", bufs=4) as xtp,
            tc.tile_pool(name="esb", bufs=4) as ep,
            tc.tile_pool(name="work", bufs=2) as wkp,
            tc.tile_pool(name="pa", bufs=4, space="PSUM") as pa,
            tc.tile_pool(name="ps", bufs=2, space="PSUM") as pspool,
            tc.tile_pool(name="pz", bufs=1, space="PSUM") as pzpool,
            tc.tile_pool(name="pb", bufs=1, space="PSUM") as pbpool,
            tc.tile_pool(name="dram", bufs=1, space="DRAM") as dp,
        ):
            # ---- persistent SBUF tiles ----
            wq_sb = [pp.tile([128, WCOL], F32R, tag=f"wq{i}", name=f"wq{i}") for i in range(DM_T)]
            wk_sb = [pp.tile([128, WCOL], F32R, tag=f"wk{i}", name=f"wk{i}") for i in range(DM_T)]
            wv_sb = [pp.tile([128, WCOL], F32R, tag=f"wv{i}", name=f"wv{i}") for i in range(DM_T)]
            wo_sb = [pp.tile([128, DM], F32R, tag=f"wo{i}", name=f"wo{i}") for i in range(DM_T)]
            qt_sb = [pp.tile([128, S], F32R, tag=f"qt{t}", name=f"qt{t}") for t in range(2)]
            kt_sb = [pp.tile([128, S], F32R, tag=f"kt{t}", name=f"kt{t}") for t in range(2)]
            vaug = [pp.tile([128, H_LOC * (DH + 1)], F32R, tag=f"va{k}", name=f"va{k}")
                    for k in range(S_T)]
            zt_sb = [pp.tile([128, S], F32R, tag=f"zt{t}", name=f"zt{t}") for t in range(2)]
            ztf = [pp.tile([128, 256], F32R, tag=f"zf{i}", name=f"zf{i}")
                   for i in range(2 * DM_T)]
            bq_sb = [pp.tile([128, 1], F32, tag=f"bq{t}", name=f"bq{t}") for t in range(2)]
            bk_sb = [pp.tile([128, 1], F32, tag=f"bk{t}", name=f"bk{t}") for t in range(2)]
            bvb_sb = pp.tile([128, H_LOC * (DH + 1)], F32R, tag="bvb")
            bob_sb = pp.tile([128, DM], F32, tag="bob")
            maskt_sb = pp.tile([128, S_T], F32, tag="maskt")
            tri_sb = pp.tile([128, 896], F32, tag="tri")
            ones_sb = pp.tile([1, DH], F32R, tag="ones")

            # ---- load constants/weights ----
            for i in range(DM_T):
                nc.sync.dma_start(wq_sb[i], w_q[128 * i:128 * (i + 1), :])
                nc.sync.dma_start(wk_sb[i], w_k[128 * i:128 * (i + 1), :])
                nc.sync.dma_start(wv_sb[i], w_v[128 * i:128 * (i + 1), :])
                nc.sync.dma_start(wo_sb[i], w_o[128 * i:128 * (i + 1), :])
            for t in range(2):
                nc.sync.dma_start(bq_sb[t], bq[128 * t:128 * (t + 1), :])
                nc.sync.dma_start(bk_sb[t], bk[128 * t:128 * (t + 1), :])
            nc.sync.dma_start(bvb_sb, bvb[:, :])
            nc.sync.dma_start(bob_sb, bob[:, :])
            nc.sync.dma_start(maskt_sb, maskt[:, :])
            nc.sync.dma_start(tri_sb, tri[:, :])
            nc.sync.dma_start(ones_sb, ones64[:, :])

            # ---- Q/K projections (transposed layout) ----
            # QT[wcol, x] = sum_dm W_Q[dm, wcol] * X[x, dm]
            for xc in range(S_C):
                xq_t = [xtp.tile([128, 512], F32R, tag="xq", name="xq") for _ in range(DM_T)]
                xk_t = [xtp.tile([128, 512], F32R, tag="xk", name="xk") for _ in range(DM_T)]
                psq = [pa.tile([128, 512], F32, tag="pa", name="pa") for _ in range(2)]
                psk = [pa.tile([128, 512], F32, tag="pa", name="pa") for _ in range(2)]
                for dm in range(DM_T):
                    nc.sync.dma_start(
                        xq_t[dm], xt_q[128 * dm:128 * (dm + 1), 512 * xc:512 * (xc + 1)])
                    nc.sync.dma_start(
                        xk_t[dm], xt_k[128 * dm:128 * (dm + 1), 512 * xc:512 * (xc + 1)])
                    for wc in range(2):
                        nc.tensor.matmul(
                            psq[wc], wq_sb[dm][:, 128 * wc:128 * (wc + 1)],
                            xq_t[dm], start=(dm == 0), stop=(dm == DM_T - 1))
                        nc.tensor.matmul(
                            psk[wc], wk_sb[dm][:, 128 * wc:128 * (wc + 1)],
                            xk_t[dm], start=(dm == 0), stop=(dm == DM_T - 1))
                for wc in range(2):
                    nc.scalar.activation(
                        qt_sb[wc][:, 512 * xc:512 * (xc + 1)], psq[wc],
                        mybir.ActivationFunctionType.Identity, bias=bq_sb[wc])
                    nc.scalar.activation(
                        kt_sb[wc][:, 512 * xc:512 * (xc + 1)], psk[wc],
                        mybir.ActivationFunctionType.Identity, bias=bk_sb[wc])

            # ---- V projection (natural layout + ones column per head) ----
            for xc in range(S_C):
                xv_t = [xtp.tile([128, 512], F32R, tag="xv", name="xv") for _ in range(DM_T)]
                psv = [pa.tile([128, WCOL], F32, tag="pa", name="pav") for _ in range(4)]
                for dm in range(DM_T):
                    nc.sync.dma_start(
                        xv_t[dm], xt_v[128 * dm:128 * (dm + 1), 512 * xc:512 * (xc + 1)])
                    for x4 in range(4):
                        nc.tensor.matmul(
                            psv[x4], xv_t[dm][:, 128 * x4:128 * (x4 + 1)],
                            wv_sb[dm], start=(dm == 0), stop=(dm == DM_T - 1))
                for x4 in range(4):
                    ki = 4 * xc + x4
                    va3 = vaug[ki].rearrange("p (h x) -> p h x", h=H_LOC)
                    bvb3 = bvb_sb.rearrange("p (h x) -> p h x", h=H_LOC)
                    nc.vector.scalar_tensor_tensor(
                        va3[:, :, 0:DH],
                        psv[x4].rearrange("p (h d) -> p h d", h=H_LOC),
                        1.0, bvb3[:, :, 0:DH],
                        op0=mybir.AluOpType.mult, op1=mybir.AluOpType.add)
                    nc.vector.tensor_copy(
                        va3[:, :, DH:DH + 1], bvb3[:, :, DH:DH + 1])

            # ---- causal attention, scores transposed [k, q] ----
            for h in range(H_LOC):
                th, ho = h // 2, 64 * (h % 2)
                for c in range(S_C):
                    kmax = 4 * c + 4  # k tiles 0..kmax-1 (rest fully masked)
                    psz = pzpool.tile([DH + 1, 512], F32, tag="pz")
                    for ki in range(kmax):
                        pss = pspool.tile([128, 512], F32, tag="ps")
                        nc.tensor.matmul(
                            pss,
                            kt_sb[th][ho:ho + DH, 128 * ki:128 * (ki + 1)],
                            qt_sb[th][ho:ho + DH, 512 * c:512 * (c + 1)],
                            start=True, stop=True)
                        j = ki - 4 * c
                        if j >= 0:  # diagonal block: add triangle mask
                            nc.vector.tensor_add(
                                pss, pss, tri_sb[:, 384 - 128 * j:896 - 128 * j])
                        esb = ep.tile([128, 512], F32R, tag="e")
                        nc.scalar.activation(
                            esb, pss, mybir.ActivationFunctionType.Exp,
                            bias=maskt_sb[:, ki:ki + 1], scale=float(SCALE))
                        nc.tensor.matmul(
                            psz, vaug[ki][:, (DH + 1) * h:(DH + 1) * (h + 1)],
                            esb, start=(ki == 0), stop=(ki == kmax - 1))
                    # normalize: recip of denominator row, broadcast via K=1 mm
                    recip = wkp.tile([1, 512], F32R, tag="recip")
                    with nc.allow_low_precision(reason="f32r softmax denom"):
                        nc.vector.reciprocal(recip, psz[DH:DH + 1, :])
                    psb = pbpool.tile([DH, 512], F32, tag="pb")
                    nc.tensor.matmul(psb, ones_sb, recip,
                                     start=True, stop=True)
                    zraw = wkp.tile([DH, 512], F32, tag="zraw")
                    nc.scalar.copy(zraw, psz[0:DH, :])
                    nc.vector.tensor_mul(
                        zt_sb[th][ho:ho + DH, 512 * c:512 * (c + 1)], zraw, psb)

            # ---- AllToAll over all 8 cores ----
            # Core c owns q rows [256c, 256c+256) of BOTH batches after the
            # exchange; my shard j = my heads' z^T for q cols [256j, 256j+256)
            # of my batch. Received slot p = peer p's 4 heads (batch p//4)
            # for my 256 q rows. Slots 0-3 stack to all 16 heads of batch 0,
            # slots 4-7 to batch 1.
            a2a_in = dp.tile([N_CORES * 2 * 128, 256], F32R, tag="a2a_in")
            a2a_out = dp.tile([N_CORES * 2 * 128, 256], F32R, tag="a2a_out")
            for t in range(2):
                for j in range(N_CORES):
                    nc.sync.dma_start(
                        a2a_in[256 * j + 128 * t:256 * j + 128 * (t + 1), :],
                        zt_sb[t][:, 256 * j:256 * (j + 1)])
            nc.gpsimd.collective_compute(
                "AllToAll", mybir.AluOpType.bypass,
                replica_groups=[[0, 1, 2, 3, 4, 5, 6, 7]],
                ins=[a2a_in.opt()], outs=[a2a_out.opt()])
            for bh in range(2):
                for i in range(DM_T):
                    nc.sync.dma_start(
                        ztf[DM_T * bh + i],
                        a2a_out[1024 * bh + 128 * i:1024 * bh + 128 * (i + 1), :])

            # ---- output projection: 256 q rows per batch ----
            for bh in range(2):
                for qt in range(2):
                    osb = wkp.tile([128, DM], F32, tag="osb")
                    for mc in range(2):
                        pso = pa.tile([128, 512], F32, tag="pa", name="pso")
                        for hd in range(DM_T):
                            nc.tensor.matmul(
                                pso,
                                ztf[DM_T * bh + hd][:, 128 * qt:128 * (qt + 1)],
                                wo_sb[hd][:, 512 * mc:512 * (mc + 1)],
                                start=(hd == 0), stop=(hd == DM_T - 1))
                        nc.vector.tensor_add(
                            osb[:, 512 * mc:512 * (mc + 1)], pso,
                            bob_sb[:, 512 * mc:512 * (mc + 1)])
                    nc.sync.dma_start(
                        out[256 * bh + 128 * qt:256 * bh + 128 * (qt + 1), :], osb)

    nc.finalize()
    return nc


_NC = None


def _get_nc():
    global _NC
    if _NC is None:
        _NC = build_bass()
    return _NC


def make_in_maps(query_input, key_input, value_input, additive_attention_mask,
                 W_Q, W_K, W_V, W_O, b_Q, b_K, b_V, b_O):
    f = np.float32
    tri = np.where(
        np.arange(896, dtype=np.int64)[None, :] - 384
        >= np.arange(128, dtype=np.int64)[:, None],
        f(0.0), f(MASK_VAL)).astype(f)
    bob = np.ascontiguousarray(np.broadcast_to(b_O.astype(f), (128, DM)))
    wo = np.ascontiguousarray(W_O.astype(f).reshape(DM, DM))
    in_maps = []
    for c in range(N_CORES):
        b, rk = c // GROUP, c % GROUP
        hs = slice(H_LOC * rk, H_LOC * (rk + 1))
        wq = np.ascontiguousarray(
            W_Q[hs].astype(f).transpose(1, 0, 2).reshape(DM, WCOL))
        wk = np.ascontiguousarray(
            W_K[hs].astype(f).transpose(1, 0, 2).reshape(DM, WCOL))
        wv = np.ascontiguousarray(
            W_V[hs].astype(f).transpose(1, 0, 2).reshape(DM, WCOL))
        bvb = np.zeros((128, H_LOC * (DH + 1)), f)
        for h in range(H_LOC):
            bvb[:, (DH + 1) * h:(DH + 1) * h + DH] = b_V[H_LOC * rk + h].astype(f)
            bvb[:, (DH + 1) * h + DH] = 1.0
        in_maps.append({
            "xt_q": np.ascontiguousarray(query_input[b].astype(f).T),
            "xt_k": np.ascontiguousarray(key_input[b].astype(f).T),
            "xt_v": np.ascontiguousarray(value_input[b].astype(f).T),
            "w_q": wq, "w_k": wk, "w_v": wv, "w_o": wo,
            "bq": np.ascontiguousarray(b_Q[hs].astype(f).reshape(WCOL, 1)),
            "bk": np.ascontiguousarray(b_K[hs].astype(f).reshape(WCOL, 1)),
            "bvb": bvb, "bob": bob,
            "ones64": np.ones((1, DH), f),
            "maskt": np.ascontiguousarray(
                additive_attention_mask[b, 0, 0].astype(f).reshape(S_T, 128).T),
            "tri": tri,
        })
    return in_maps


def assemble_output(results):
    out = np.empty((B, S, DM), np.float32)
    for c in range(N_CORES):
        out[0, 256 * c:256 * (c + 1), :] = results[c]["out"][:256]
        out[1, 256 * c:256 * (c + 1), :] = results[c]["out"][256:]
    return out


def kernel(**inputs):
    nc = _get_nc()
    in_maps = make_in_maps(**inputs)
    res = run_bass_kernel_spmd(nc, in_maps, core_ids=list(range(N_CORES)))
    return assemble_output(res.results)
